# revision 33
# baseline (speedup 1.0000x reference)
"""Trainium2 Bass kernel for nn_Attention_44195213476226 (coverage attention).

Reference math (B=32, S=1024, H=512, D=2H=1024):
    s_t      = concat(h_dec, c_dec)            # (B,1,D)
    dec_feat = s_t @ Ws_w.T + Ws_b             # (B,1,D)
    enc_feat = E @ Wh_w.T                      # (B,S,D)
    cov_feat = cov[...,None] * Wc_w[:,0]       # (B,S,D)
    score    = (enc_feat+dec_feat+cov_feat)@v  # (B,S)
    w        = renorm(softmax(score)*mask)
    ctx      = w @ E ; cov_new = cov + w

The score factorizes:  score[b,s] = E[b,s,:]@u + alpha[b] + beta*cov[b,s]
with u = v @ Wh (a (D,) vector), alpha[b] = dec_feat[b]@v, beta = v@Wc.
alpha[b] is constant across s and softmax is shift-invariant, so alpha
cannot affect any output.  The kernel folds diag(u) into the encoder
activations (E16 = fp16(E*u), so the device-side score is a plain row sum)
and folds beta*cov + ln(mask) into a per-(s)-element bias added inside the
reduction, making  em[b,s] = exp(E16[b,s,:].sum() + bias[b,s])  the
complete masked unnormalized attention weight.

Device per core (data-parallel over batch, 4 batches/core on 8 cores):
  - stream the 8 MB E16 shard over all three DMA queues (sync/scalar HWDGE
    + gpsimd SWDGE) so the transfers run concurrently,
  - score row-sums as single fused ops: DVE tensor_scalar with accum_out
    (runs in the 4x DVE mode on fp16) with the bias folded in via scalar2,
    plus a gpsimd half-add prepass on part of the tiles,
  - exp on ACT (fp16 out), then the em columns are packed into per-batch
    zero-padded stationaries so the context matmuls for all 4 batches
    accumulate into one [4, 512] PSUM tile per 512-wide output half
    (the tensor engine streams 64 fp16 matmuls, em stationary / E moving),
  - outputs: em16 (the unnormalized masked weights) and the unnormalized
    context accumulators.  The host applies the scalar normalizers
    (1/Z_b, 1/u_d) exactly as flash-attention does with its (acc, l) pair.
"""

import numpy as np

B, S, H = 32, 1024, 512
D = 2 * H
NCORES = 8
BLOC = B // NCORES        # batches per core
ST = S // 128             # s-tiles of 128 rows per batch
NWARM = 7                 # PE p-state warmup matmuls

# score-reduce engine assignment: (b, i) in POOL_ASSIST gets a gpsimd
# half-add prepass + DVE half-width reduce; everything else is a DVE
# fused reduce (tensor_scalar + accum_out, 4x mode).
POOL_ASSIST = {(1, 6), (1, 7), (2, 6), (2, 7), (3, 6), (3, 7)}

# E16 DMA chunks (batch, tile_lo, tile_hi) per queue.  The sync and gpsimd
# chunks are all issued up front; the scalar-queue chunk for batch b+1 is
# emitted after batch b's exps so the ACT queue alternates DMA issue with
# compute instead of serializing all transfers first.
SYNC_CHUNKS = [(0, 0, 2), (0, 2, 3), (1, 0, 3), (2, 0, 3), (3, 0, 3)]
GPSIMD_CHUNKS = [(0, 6, 8), (1, 6, 8), (2, 6, 8), (3, 3, 8)]
SCALAR_CHUNKS = {-1: (0, 3, 6), 0: (1, 3, 6), 1: (2, 3, 6)}

_CACHE = {}


def _build_bass():
    import concourse.bass as bass
    import concourse.mybir as mybir
    from concourse import tile
    from contextlib import ExitStack

    fp32 = mybir.dt.float32
    fp16 = mybir.dt.float16
    ALU = mybir.AluOpType
    ACTF = mybir.ActivationFunctionType

    nc = bass.Bass()

    e_d = nc.dram_tensor("e16", [BLOC, S, D], fp16, kind="ExternalInput")
    # bias/1024 and bias/512 (bias = beta*cov + ln(mask)), partition layout
    bc1024_d = nc.dram_tensor("bc1024", [128, BLOC, ST], fp32, kind="ExternalInput")
    bc512_d = nc.dram_tensor("bc512", [128, BLOC, ST], fp32, kind="ExternalInput")
    em_d = nc.dram_tensor("emout", [128, BLOC, ST], fp16, kind="ExternalOutput")
    ctx_d = nc.dram_tensor("ctxr", [BLOC, 2, 512], fp32, kind="ExternalOutput")

    with tile.TileContext(nc) as tc, ExitStack() as ctx:
        const = ctx.enter_context(tc.tile_pool(name="const", bufs=1))
        epool = ctx.enter_context(tc.tile_pool(name="epool", bufs=1))
        spool = ctx.enter_context(tc.tile_pool(name="scr", bufs=2))
        small = ctx.enter_context(tc.tile_pool(name="small", bufs=1))
        psp = ctx.enter_context(tc.tile_pool(name="ps", bufs=1, space="PSUM"))

        # --- consts / warmup fodder ---
        actdum = const.tile([128, 1], fp32, name="actdum")
        nc.gpsimd.memset(actdum[:], 0.0)
        wdum = const.tile([128, 4], fp16, name="wdum")
        nc.vector.memset(wdum[:], 0.0)
        mdum = const.tile([128, 512], fp16, name="mdum")
        nc.vector.memset(mdum[:], 0.0)
        w16pad = []
        for b in range(BLOC):
            wp = const.tile([128, ST, BLOC], fp16, name=f"w16pad{b}")
            nc.vector.memset(wp[:], 0.0)
            w16pad.append(wp)

        # ACT: load the exp_and_others table early; bias passed as a
        # zeros-AP so no framework const-AP dependency sneaks in
        actdum2 = const.tile([128, 1], fp32, name="actdum2")
        nc.scalar.activation(actdum2[:], actdum[:], ACTF.Exp, bias=actdum[:, 0:1])

        # PE p-state warmup: keep the PE continuously busy from t~0 so the
        # clock ramp completes right as the first real matmul's deps resolve
        cpsd = psp.tile([4, 512], fp32, name="cpsd")
        for n in range(NWARM):
            nc.tensor.matmul(cpsd[:], wdum[:], mdum[:], start=True, stop=True)

        # --- E16 loads ---
        e16 = epool.tile([128, BLOC, ST, D], fp16, name="e16_t")

        def echunk(eng, b, lo, hi):
            eng.dma_start(
                e16[:, b, lo:hi, :],
                e_d[b, lo * 128:hi * 128, :].rearrange("(i p) d -> p i d", p=128),
            )

        bc1024 = const.tile([128, BLOC, ST], fp32, name="bc1024_t")
        bc512 = const.tile([128, BLOC, ST], fp32, name="bc512_t")

        nc.gpsimd.dma_start(bc1024[:], bc1024_d[:])
        nc.gpsimd.dma_start(bc512[:], bc512_d[:])
        for ch in SYNC_CHUNKS:
            echunk(nc.sync, *ch)
        for ch in GPSIMD_CHUNKS:
            echunk(nc.gpsimd, *ch)
        echunk(nc.scalar, *SCALAR_CHUNKS[-1])

        rawq = {(b, q): small.tile([128, 2], fp32, name=f"raw{b}_{q}")
                for b in range(BLOC) for q in range(ST // 2)}
        em16 = small.tile([128, BLOC, ST], fp16, name="em16")
        ctxs = small.tile([BLOC, D], fp32, name="ctxs")
        cps = [psp.tile([4, 512], fp32, name=f"cps{h}") for h in range(2)]

        # --- score reduces + exp + stationary fills, batch by batch ---
        for b in range(BLOC):
            for i in range(ST):
                et = e16[:, b, i, :]
                if (b, i) in POOL_ASSIST:
                    # gpsimd half-add prepass, then DVE half-width reduce
                    half = spool.tile([128, 512], fp16, name="scrP", tag="scrP", bufs=2)
                    nc.gpsimd.tensor_tensor(half[:], et[:, :512], et[:, 512:], ALU.add)
                    scr = spool.tile([128, 512], fp16, name="scrPd", tag="scrPd", bufs=2)
                    nc.vector.tensor_scalar(
                        scr[:], half[:], 1.0, bc512[:, b, i:i + 1],
                        ALU.mult, ALU.add,
                        accum_out=rawq[b, i // 2][:, i % 2:i % 2 + 1])
                else:
                    scr = spool.tile([128, D], fp16, name="scrD", tag="scrD", bufs=2)
                    nc.vector.tensor_scalar(
                        scr[:], et, 1.0, bc1024[:, b, i:i + 1],
                        ALU.mult, ALU.add,
                        accum_out=rawq[b, i // 2][:, i % 2:i % 2 + 1])
                if i % 2 == 1:
                    q = i // 2
                    nc.scalar.activation(
                        em16[:, b, 2 * q:2 * q + 2], rawq[b, q][:],
                        ACTF.Exp, bias=actdum[:, 0:1])
                    nc.vector.tensor_scalar(
                        w16pad[b][:, 2 * q:2 * q + 2, b],
                        em16[:, b, 2 * q:2 * q + 2], 1.0, None, ALU.mult)

            # --- context matmuls for this batch: all into the shared [4,512]
            # PSUM tiles (batch b owns stationary column b) ---
            for h in range(2):
                for i in range(ST):
                    nc.tensor.matmul(
                        cps[h][:], w16pad[b][:, i, :],
                        e16[:, b, i, h * 512:(h + 1) * 512],
                        start=(b == 0 and i == 0), stop=(b == BLOC - 1 and i == ST - 1))

            # em16 for this batch streams out early (host derives w, Z, cov)
            nc.gpsimd.dma_start(em_d[:, b, :], em16[:, b, :])
            # next scalar-queue E chunk goes out between this batch's exps
            # and the next batch's (keeps the ACT queue compute-responsive)
            if b in SCALAR_CHUNKS:
                echunk(nc.scalar, *SCALAR_CHUNKS[b])


        # --- PSUM -> SBUF copies (plain; host applies 1/Z and 1/u).  h0
        # overlaps the tail of the PE stream; h1 (the critical tail) is
        # split into two quarter copies + DMAs running on parallel engines
        # and queues ---
        nc.scalar.activation(ctxs[:, 0:512], cps[0][:], ACTF.Copy)
        nc.sync.dma_start(ctx_d[:, 0, :], ctxs[:, 0:512])
        nc.scalar.activation(ctxs[:, 512:768], cps[1][:, 0:256], ACTF.Copy)
        nc.vector.tensor_scalar(ctxs[:, 768:1024], cps[1][:, 256:512],
                                1.0, None, ALU.mult)
        nc.scalar.dma_start(ctx_d[:, 1, 0:256], ctxs[:, 512:768])
        nc.sync.dma_start(ctx_d[:, 1, 256:512], ctxs[:, 768:1024])

    _legalize_sync_waits(nc, mybir)
    return nc


def _legalize_sync_waits(nc, mybir):
    """The walrus build in this container allows only ONE embedded sync-wait
    per instruction ("Too many sync wait commands" otherwise).  Tile emits
    up to three.  Fix: hoist the excess waits, ordering fully preserved,
    into standalone InstEventSemaphore instructions (the same type the
    framework barriers use) immediately before the instruction on the same
    engine queue."""
    wid = 0
    for fn in nc.m.functions:
        for blk in fn.blocks:
            new = []
            for inst in blk.instructions:
                si = inst.sync_info
                if si is not None and si.on_wait:
                    waits = list(si.on_wait)
                    while len(waits) > 1:
                        w = waits.pop(0)
                        wid += 1
                        ev = mybir.InstEventSemaphore(
                            name=f"I-hoistw-{wid}",
                            engine=inst.engine,
                            ins=[],
                            outs=[],
                            sync_info=mybir.SyncInfo(on_wait=[w], on_update=[]),
                        )
                        nc.register_instruction(ev, overwrite=True)
                        new.append(ev)
                    inst.sync_info = mybir.SyncInfo(
                        on_wait=waits, on_update=list(si.on_update)
                    )
                new.append(inst)
            blk.instructions[:] = new


def _get_nc():
    if "nc" not in _CACHE:
        _CACHE["nc"] = _build_bass()
    return _CACHE["nc"]


def _prep_inputs(inputs):
    E = np.asarray(inputs["encoder_output"], dtype=np.float32)
    mask = np.asarray(inputs["x_padding_masks"], dtype=np.float64)
    cov = np.asarray(inputs["coverage_vector"], dtype=np.float64)
    Wh = np.asarray(inputs["Wh_w"], dtype=np.float64)
    Wc = np.asarray(inputs["Wc_w"], dtype=np.float64)
    v = np.asarray(inputs["v_w"], dtype=np.float64)

    u = v[0] @ Wh                        # u[d] = sum_e v[e] * Wh[e,d]
    beta = float(v[0] @ Wc[:, 0])

    e16 = (E * u[None, None, :].astype(np.float32)).astype(np.float16)

    with np.errstate(divide="ignore"):
        bias = beta * cov + np.log(mask)          # (B,S); -inf where masked
    # (B,S) -> (128,B,ST) partition layout: x[p,b,t] = x[b, t*128+p]
    biasp = bias.reshape(B, ST, 128).transpose(2, 0, 1).astype(np.float32)

    in_maps = []
    for c in range(NCORES):
        lo, hi = c * BLOC, (c + 1) * BLOC
        in_maps.append({
            "e16": np.ascontiguousarray(e16[lo:hi]),
            "bc1024": np.ascontiguousarray(biasp[:, lo:hi] / 1024.0),
            "bc512": np.ascontiguousarray(biasp[:, lo:hi] / 512.0),
        })
    _CACHE["u"] = u
    _CACHE["cov"] = cov
    return in_maps


def _assemble(results):
    u = _CACHE["u"]
    cov = _CACHE["cov"]
    em = np.concatenate(
        [np.asarray(r["emout"], np.float64).reshape(128, BLOC, ST)
         .transpose(1, 2, 0).reshape(BLOC, S)
         for r in results], axis=0)                     # (B,S) = exp(score)*mask
    ctxr = np.concatenate(
        [np.asarray(r["ctxr"], np.float64).reshape(BLOC, D) for r in results],
        axis=0)                                                       # (B,D)

    Z = em.sum(axis=1, keepdims=True)
    w = em / Z
    covn = cov + w
    context = ctxr / (Z * u[None, :])
    return (context.astype(np.float32), w.astype(np.float32),
            covn.astype(np.float32))


def run(inputs, trace=False, **kwargs):
    """Run the Bass kernel on the 8 cores; returns ((ctx, w, cov_new), results_obj)."""
    from concourse.bass_utils import run_bass_kernel_spmd

    nc = _get_nc()
    in_maps = _prep_inputs(inputs)
    res = run_bass_kernel_spmd(nc, in_maps, list(range(NCORES)), trace=trace, **kwargs)
    return _assemble(res.results), res


def kernel(**inputs):
    out, _ = run(inputs)
    return out
